# revision 1
# baseline (speedup 1.0000x reference)
"""Trainium2 Bass kernel for complex multi-head attention (8 NeuronCores).

Sharding: core c handles batch b = c//2 and head-group g = c%2 (8 of 16
heads, i.e. 512 of 1024 embed dims). No device collectives: each core
produces a partial out-projection (its head-group's contribution) and the
host sums the two partials per batch and adds the output bias.

Per-core dataflow (all matmuls bf16 with fp32 PSUM accumulation):
  - Q/K projections computed transposed (head-dim on partitions), written
    into stacked score operands:  R = [qr; qi], KA = [kr; ki], KB = [ki; -kr]
    so one K=128 matmul per tile yields attn_real^T (KA.T @ R) and
    -attn_imag^T (KB.T @ R) with q on the free axis.
  - softmax (no max-subtraction; exp args are <= ~4 here): Square on ACT over
    the combined [real|imag] PSUM tile, bf16 add on DVE, in-place Sqrt, Exp.
  - V projected in natural layout into VRIO = [vr | vi | ones]; the AV matmul
    (lhsT = probs tile) then yields [att_r | att_i | rowsum] per q-tile, and
    the evacuation multiplies by 1/rowsum (per-partition scalar).
  - att is PE-transposed (head dim back onto partitions) for the output
    projection, which writes the fp32 partial to DRAM.
"""

import os
import sys

for _p in ("/opt/trn_rl_repo", "/root/.axon_site/_ro/trn_rl_repo"):
    if os.path.isdir(_p) and _p not in sys.path:
        sys.path.append(_p)

import numpy as np
import ml_dtypes

bf16 = ml_dtypes.bfloat16

P = 128
S = 1024
E = 1024
DL = 512  # local (per-core) head dims: 8 heads x 64
D = 64
HLOC = 8
SCALE = D ** -0.5

_NC_CACHE = None


def _build():
    import concourse.tile as tile
    from concourse import bacc, mybir
    from concourse.masks import make_identity

    f32 = mybir.dt.float32
    b16 = mybir.dt.bfloat16
    Alu = mybir.AluOpType
    Act = mybir.ActivationFunctionType

    nc = bacc.Bacc("TRN2", target_bir_lowering=False, debug=False, num_devices=8)

    def din(name, shape, dt):
        return nc.dram_tensor(name, shape, dt, kind="ExternalInput").ap()

    x_in = {n: din(n, [E, S], b16)
            for n in ("xq_r", "xq_i", "xk_r", "xk_i", "xv_r", "xv_i")}
    w_in = {n: din(n, [E, DL], b16)
            for n in ("wq_r", "wq_i", "wk_r", "wk_i", "wv_r", "wv_i")}
    wo_in = {n: din(n, [DL, E], b16) for n in ("wo_r", "wo_i")}
    bqk_in = {n: din(n, [P, 4], f32)
              for n in ("bq_r", "bq_i", "bk_r", "bk_i")}
    bv_in = {n: din(n, [P, DL], f32) for n in ("bv_r", "bv_i")}
    out_d = {n: nc.dram_tensor(n, [S, E], f32, kind="ExternalOutput").ap()
             for n in ("out_r", "out_i")}

    with tile.TileContext(nc) as tc:
        with (
            tc.tile_pool(name="persist", bufs=1) as persist,
            tc.tile_pool(name="psum", bufs=2, space="PSUM") as psp,
            tc.tile_pool(name="recp", bufs=4) as recp,
        ):
            R_st = persist.tile([P, HLOC, S], b16, tag="R")
            KA_st = persist.tile([P, HLOC, S], b16, tag="KA")
            KB_st = persist.tile([P, HLOC, S], b16, tag="KB")
            VRIO = persist.tile([P, 8, HLOC, 130], b16, tag="VRIO")
            Att = persist.tile([P, 8, 1024], b16, tag="Att")
            AttT = persist.tile([P, 8, S], b16, tag="AttT")
            ident = persist.tile([P, P], b16, tag="ident")
            make_identity(nc, ident[:])
            nc.vector.memset(VRIO[:, :, :, 128:130], 1.0)

            bias_sb = {}
            for n, ap in bqk_in.items():
                t = persist.tile([P, 4], f32, tag=n)
                nc.sync.dma_start(t[:], ap)
                bias_sb[n] = t
            for n, ap in bv_in.items():
                t = persist.tile([P, DL], f32, tag=n)
                nc.sync.dma_start(t[:], ap)
                bias_sb[n] = t
            wo_sb = {}
            for n, ap in wo_in.items():
                t = persist.tile([P, 4, E], b16, tag=n)
                nc.sync.dma_start(t[:], ap.rearrange("(dc p) o -> p dc o", p=P))
                wo_sb[n] = t

            # ---- Phase 1: Q/K projections (transposed out) + stacking evac
            # evac entries: (engine, dest_tile, dest_base, bias_name, scale)
            qk_projs = [
                ("xq_r", "wq_r", [("dve", R_st, 0, "bq_r", 1.0)]),
                ("xq_i", "wq_i", [("dve", R_st, 64, "bq_i", 1.0)]),
                ("xk_r", "wk_r", [("dve", KA_st, 0, "bk_r", 1.0),
                                  ("dve", KB_st, 64, "bk_r", -1.0)]),
                ("xk_i", "wk_i", [("dve", KA_st, 64, "bk_i", 1.0),
                                  ("dve", KB_st, 0, "bk_i", 1.0)]),
            ]
            v_projs = [("xv_r", "wv_r", "bv_r", 0), ("xv_i", "wv_i", "bv_i", 1)]

            with tc.tile_pool(name="stream", bufs=2) as stream:
                for xn, wn, evacs in qk_projs:
                    x_sb = stream.tile([P, 8, S], b16, tag="x")
                    nc.sync.dma_start(
                        x_sb[:], x_in[xn].rearrange("(eo p) s -> p eo s", p=P))
                    w_sb = stream.tile([P, 8, DL], b16, tag="w")
                    nc.sync.dma_start(
                        w_sb[:], w_in[wn].rearrange("(eo p) d -> p eo d", p=P))
                    for dt in range(4):
                        ps = psp.tile([P, S], f32, tag="big")
                        for eo in range(8):
                            for nh in range(2):
                                nc.tensor.matmul(
                                    ps[:, nh * 512:(nh + 1) * 512],
                                    w_sb[:, eo, dt * P:(dt + 1) * P],
                                    x_sb[:, eo, nh * 512:(nh + 1) * 512],
                                    start=(eo == 0), stop=(eo == 7))
                        for half in range(2):
                            h = 2 * dt + half
                            src = ps[64 * half:64 * half + 64, :]
                            for eng, dest, base, bn, sc in evacs:
                                b_ap = bias_sb[bn][64 * half:64 * half + 64,
                                                   dt:dt + 1]
                                dst = dest[base:base + 64, h, :]
                                if sc == 1.0:
                                    nc.vector.tensor_scalar(
                                        dst, src, b_ap, None, op0=Alu.add)
                                else:
                                    nc.vector.tensor_scalar(
                                        dst, src, b_ap, sc,
                                        op0=Alu.add, op1=Alu.mult)

                # ---- Phase 2: V projections (natural out) into VRIO
                for xn, wn, bn, ri in v_projs:
                    x_sb = stream.tile([P, 8, S], b16, tag="x")
                    nc.sync.dma_start(
                        x_sb[:], x_in[xn].rearrange("(eo p) s -> p eo s", p=P))
                    w_sb = stream.tile([P, 8, DL], b16, tag="w")
                    nc.sync.dma_start(
                        w_sb[:], w_in[wn].rearrange("(eo p) d -> p eo d", p=P))
                    for st in range(8):
                        ps = psp.tile([P, DL], f32, tag="med")
                        for eo in range(8):
                            nc.tensor.matmul(
                                ps[:], x_sb[:, eo, st * P:(st + 1) * P],
                                w_sb[:, eo, :],
                                start=(eo == 0), stop=(eo == 7))
                        nc.vector.tensor_tensor(
                            VRIO[:, st, :, ri * D:(ri + 1) * D],
                            ps[:].rearrange("p (h d) -> p h d", h=HLOC),
                            bias_sb[bn][:].rearrange("p (h d) -> p h d", h=HLOC),
                            Alu.add)

            # ---- Phase 3: attention per head
            with (
                tc.tile_pool(name="sqp", bufs=3) as sqp,
                tc.tile_pool(name="s2p", bufs=9) as s2p,
                tc.tile_pool(name="ptp", bufs=16) as ptp,
            ):
                for h in range(HLOC):
                    s2s = []
                    for kc in range(8):
                        s2 = s2p.tile([P, S], b16, tag="s2")
                        for qh in range(2):
                            ps = psp.tile([P, S], f32, tag="big")
                            nc.tensor.matmul(
                                ps[:, 0:512],
                                KA_st[:, h, kc * P:(kc + 1) * P],
                                R_st[:, h, qh * 512:(qh + 1) * 512],
                                start=True, stop=True)
                            nc.tensor.matmul(
                                ps[:, 512:1024],
                                KB_st[:, h, kc * P:(kc + 1) * P],
                                R_st[:, h, qh * 512:(qh + 1) * 512],
                                start=True, stop=True)
                            sq = sqp.tile([P, S], b16, tag="sq")
                            nc.scalar.activation(sq[:], ps[:], Act.Square)
                            nc.vector.tensor_tensor(
                                s2[:, qh * 512:(qh + 1) * 512],
                                sq[:, 0:512], sq[:, 512:1024], Alu.add)
                        s2s.append(s2)
                    for kc in range(8):
                        nc.scalar.activation(s2s[kc][:], s2s[kc][:], Act.Sqrt)
                    pts = []
                    for kc in range(8):
                        pt = ptp.tile([P, S], b16, tag="pt")
                        nc.scalar.activation(
                            pt[:], s2s[kc][:], Act.Exp, scale=float(SCALE))
                        pts.append(pt)
                    for qt in range(8):
                        psa = psp.tile([P, 130], f32, tag="small")
                        for kc in range(8):
                            nc.tensor.matmul(
                                psa[:, 0:129],
                                pts[kc][:, qt * P:(qt + 1) * P],
                                VRIO[:, kc, h, 0:129],
                                start=(kc == 0), stop=(kc == 7))
                        rec = recp.tile([P, 1], f32, tag="rec")
                        nc.vector.reciprocal(rec[:], psa[:, 128:129])
                        # Att free layout: ri*512 + h*64 + d (real block, then
                        # imag block) so phase-4 transpose sources are
                        # contiguous [128,128] slices.
                        for ri in range(2):
                            nc.vector.tensor_scalar(
                                Att[:, qt, ri * 512 + h * D:
                                    ri * 512 + (h + 1) * D],
                                psa[:, ri * D:(ri + 1) * D],
                                rec[:], None, op0=Alu.mult)

            # ---- Phase 4: transpose att (head dims back onto partitions)
            with tc.tile_pool(name="fin", bufs=2) as fin:
                for qt in range(8):
                    for ch in range(8):  # 4 real chunks then 4 imag chunks
                        tp = psp.tile([P, P], b16, tag="small")
                        nc.tensor.transpose(
                            tp[:], Att[:, qt, ch * P:(ch + 1) * P], ident[:])
                        nc.vector.tensor_copy(
                            AttT[:, ch, qt * P:(qt + 1) * P], tp[:])

                # ---- Phase 5: output projections (partial, no bias)
                for ri, (wn, on) in enumerate(
                        [("wo_r", "out_r"), ("wo_i", "out_i")]):
                    for st in range(8):
                        ob = fin.tile([P, E], f32, tag="ob")
                        for oh in range(2):
                            ps = psp.tile([P, DL], f32, tag="med")
                            for dc in range(4):
                                nc.tensor.matmul(
                                    ps[:],
                                    AttT[:, ri * 4 + dc, st * P:(st + 1) * P],
                                    wo_sb[wn][:, dc, oh * 512:(oh + 1) * 512],
                                    start=(dc == 0), stop=(dc == 3))
                            nc.vector.tensor_copy(
                                ob[:, oh * 512:(oh + 1) * 512], ps[:])
                        nc.sync.dma_start(
                            out_d[on][st * P:(st + 1) * P, :], ob[:])

    nc.compile()
    return nc


def make_in_maps(inputs):
    """Shard + host-prep the full inputs into 8 per-core input maps."""
    inp = {k: np.asarray(v) for k, v in inputs.items()}
    xs = {
        "xq": ("query_real", "query_imag"),
        "xk": ("key_real", "key_imag"),
        "xv": ("value_real", "value_imag"),
    }
    per_g = []
    for g in range(2):
        rows = slice(g * DL, (g + 1) * DL)
        m = {}
        for wn, src in (("wq_r", "Wq_r"), ("wq_i", "Wq_i"),
                        ("wk_r", "Wk_r"), ("wk_i", "Wk_i"),
                        ("wv_r", "Wv_r"), ("wv_i", "Wv_i")):
            m[wn] = np.ascontiguousarray(inp[src][rows].T).astype(bf16)
        for wn, src in (("wo_r", "Wo_r"), ("wo_i", "Wo_i")):
            m[wn] = np.ascontiguousarray(inp[src][:, rows].T).astype(bf16)
        for bn, src in (("bq_r", "bq_r"), ("bq_i", "bq_i"),
                        ("bk_r", "bk_r"), ("bk_i", "bk_i")):
            m[bn] = np.ascontiguousarray(
                inp[src][rows].reshape(4, P).T).astype(np.float32)
        for bn, src in (("bv_r", "bv_r"), ("bv_i", "bv_i")):
            m[bn] = np.ascontiguousarray(
                np.broadcast_to(inp[src][rows], (P, DL))).astype(np.float32)
        per_g.append(m)

    in_maps = []
    for c in range(8):
        b, g = c // 2, c % 2
        m = dict(per_g[g])
        for pref, (re_n, im_n) in xs.items():
            m[pref + "_r"] = np.ascontiguousarray(inp[re_n][b].T).astype(bf16)
            m[pref + "_i"] = np.ascontiguousarray(inp[im_n][b].T).astype(bf16)
        in_maps.append(m)
    return in_maps


def combine_outputs(results, inputs):
    bo_r = np.asarray(inputs["bo_r"], np.float32)
    bo_i = np.asarray(inputs["bo_i"], np.float32)
    B = 4
    out_r = np.empty((B, S, E), np.float32)
    out_i = np.empty((B, S, E), np.float32)
    for b in range(B):
        out_r[b] = results[2 * b]["out_r"] + results[2 * b + 1]["out_r"] + bo_r
        out_i[b] = results[2 * b]["out_i"] + results[2 * b + 1]["out_i"] + bo_i
    return out_r, out_i


def get_nc():
    global _NC_CACHE
    if _NC_CACHE is None:
        _NC_CACHE = _build()
    return _NC_CACHE


def kernel(**inputs):
    from concourse.bass_utils import run_bass_kernel_spmd

    nc = get_nc()
    in_maps = make_in_maps(inputs)
    res = run_bass_kernel_spmd(nc, in_maps, list(range(8)))
    return combine_outputs(res.results, inputs)



# revision 15
# speedup vs baseline: 151.1354x; 151.1354x over previous
"""Trainium2 Bass kernel for complex multi-head attention (8 NeuronCores).

Sharding: core c handles batch b = c//2 and head-group g = c%2 (8 of 16
heads, i.e. 512 of 1024 embed dims). No device collectives: each core
produces a partial out-projection (its head-group's contribution) and the
host sums the two partials per batch and adds the output bias.

Per-core dataflow (all matmuls bf16 with fp32 PSUM accumulation):
  - Q/K projections computed transposed (head-dim on partitions) and
    evacuated on the ACT engine (Identity + per-partition bias) into three
    stacked score operands: R = [qr; qi], R2 = [qi; -qr], KA = [kr; ki].
    One K=128 matmul pair per tile yields attn_real^T (KA.T @ R) and
    attn_imag^T (KA.T @ R2) with q on the free axis.
  - softmax (no max-subtraction; exp args are <= ~4 here), engine-balanced:
    squares (s*s, PSUM->bf16) split across DVE and GpSimd, pair-sum on DVE,
    then per head a single [128, 8192] Sqrt (scale=SCALE^2 folds the
    1/sqrt(d) factor) and a single [128, 8192] Exp on ACT. Heads are
    processed in pairs so the ACT Sqrt/Exp table loads amortize.
  - V projected in natural layout into VRIO = [vr | vi | ones]; the AV matmul
    (lhsT = probs tile) then yields [att_r | att_i | rowsum] per q-tile, and
    the evacuation multiplies by 1/rowsum (per-partition scalar).
  - att is PE-transposed (head dim back onto partitions, copies on GpSimd)
    for the output projection, which writes the fp32 partial to DRAM.
"""

import os
import sys

for _p in ("/opt/trn_rl_repo", "/root/.axon_site/_ro/trn_rl_repo"):
    if os.path.isdir(_p) and _p not in sys.path:
        sys.path.append(_p)

import numpy as np
import ml_dtypes

bf16 = ml_dtypes.bfloat16

P = 128
S = 1024
E = 1024
DL = 512  # local (per-core) head dims: 8 heads x 64
D = 64
HLOC = 8
SCALE = D ** -0.5

# Engine split for the softmax squares (GPSIMD cannot read PSUM; a DVE op
# may read at most one PSUM operand). Route A (ACT): one Square over the
# full [r|i] tile + GpSimd pair-sum add. Route B (DVE): two fused custom
# ops, t = r*r then s2 = i*i + t, one PSUM operand each. Of the 16
# (kc, qh) tiles per head, indices in ACT_SQ take route A.
ACT_SQ = frozenset((1, 4, 7, 10, 13))

_NC_CACHE = None
_DVE_SQ = {}


def _register_dve_sq_ops():
    """Register single-PSUM-operand square / square-accumulate custom DVE
    ops (out = in0^2 and out = in0^2 + in1). Idempotent."""
    if _DVE_SQ:
        return
    import numpy as np
    from concourse import dve_ops
    from concourse.dve_spec import Spec, Src0, Src1, sq, lower, _has_src1
    from concourse.dve_uop import DveOpSpec

    specs = {
        "SQ_ANT2": Spec(
            body=sq(Src0),
            reference=lambda in0, in1, s0, s1, imm2: (
                in0.astype(np.float32) ** 2)),
        "SQ_ADD_ANT2": Spec(
            body=sq(Src0) + Src1,
            reference=lambda in0, in1, s0, s1, imm2: (
                in0.astype(np.float32) ** 2 + in1)),
    }
    existing = {op.name for op in dve_ops.OPS}
    for name, spec in specs.items():
        if name in existing:
            _DVE_SQ[name] = next(o for o in dve_ops.OPS if o.name == name)
            continue
        row = max(dve_ops._SUB_OPCODE_FOR_NAME.values()) + 1
        assert row < 0x20, "custom-DVE opcode rows exhausted"
        dve_ops._SUB_OPCODE_FOR_NAME[name] = row
        shas = {}
        for ver in ("v3", "v4"):
            s = DveOpSpec(name=name, opcode=row, uops=lower(spec, ver=ver),
                          rd1_en=_has_src1(spec))
            shas[ver] = s.sha(ver)
        op = dve_ops.DveOp(name, spec, subdim=False, uops_sha=shas)
        dve_ops.OPS.append(op)
        dve_ops.CUSTOM_DVE_SPECS[name] = spec
        _DVE_SQ[name] = op


def _build():
    import concourse.tile as tile
    from concourse import bacc, mybir
    from concourse.masks import make_identity

    _register_dve_sq_ops()

    f32 = mybir.dt.float32
    b16 = mybir.dt.bfloat16
    Alu = mybir.AluOpType
    Act = mybir.ActivationFunctionType

    nc = bacc.Bacc("TRN2", target_bir_lowering=False, debug=False, num_devices=8)

    def din(name, shape, dt):
        return nc.dram_tensor(name, shape, dt, kind="ExternalInput").ap()

    x_in = {n: din(n, [E, S], b16)
            for n in ("xq_r", "xq_i", "xk_r", "xk_i", "xv_r", "xv_i")}
    w_in = {n: din(n, [E, DL], b16)
            for n in ("wq_r", "wq_i", "wk_r", "wk_i", "wv_r", "wv_i")}
    wo_in = {n: din(n, [DL, E], b16) for n in ("wo_r", "wo_i")}
    bqk_in = {n: din(n, [P, 4], f32)
              for n in ("bq_r", "bq_i", "bk_r", "bk_i", "bq_rn")}
    bv_in = {n: din(n, [P, DL], f32) for n in ("bv_r", "bv_i")}
    out_d = {n: nc.dram_tensor(n, [S, E], f32, kind="ExternalOutput").ap()
             for n in ("out_r", "out_i")}

    with tile.TileContext(nc) as tc:
        with (
            tc.tile_pool(name="persist", bufs=1) as persist,
            tc.tile_pool(name="psum", bufs=2, space="PSUM") as psp,
            tc.tile_pool(name="psmall", bufs=2, space="PSUM") as pss,
            tc.tile_pool(name="recp", bufs=4) as recp,
        ):
            R_st = persist.tile([P, HLOC, S], b16, tag="R")
            R2_st = persist.tile([P, HLOC, S], b16, tag="R2")
            KA_st = persist.tile([P, HLOC, S], b16, tag="KA")
            VRIO = persist.tile([P, 8, HLOC, 130], b16, tag="VRIO")
            Att = persist.tile([P, 8, 1024], b16, tag="Att")
            AttT = persist.tile([P, 8, S], b16, tag="AttT")
            ident = persist.tile([P, P], b16, tag="ident")
            make_identity(nc, ident[:])
            nc.vector.memset(VRIO[:, :, :, 128:130], 1.0)

            bias_sb = {}
            for n, ap in bqk_in.items():
                t = persist.tile([P, 4], f32, tag=n)
                nc.sync.dma_start(t[:], ap)
                bias_sb[n] = t
            for n, ap in bv_in.items():
                t = persist.tile([P, DL], f32, tag=n)
                nc.sync.dma_start(t[:], ap)
                bias_sb[n] = t

            # ---- Phase 1: Q/K projections (transposed out) + ACT evacs
            # evac entries: (dest_tile, dest_base, bias_name, scale)
            qk_projs = [
                ("xk_r", "wk_r", [(KA_st, 0, "bk_r", 1.0)]),
                ("xk_i", "wk_i", [(KA_st, 64, "bk_i", 1.0)]),
                ("xq_r", "wq_r", [(R_st, 0, "bq_r", 1.0),
                                  (R2_st, 64, "bq_rn", -1.0)]),
                ("xq_i", "wq_i", [(R_st, 64, "bq_i", 1.0),
                                  (R2_st, 0, "bq_i", 1.0)]),
            ]
            v_projs = [("xv_r", "wv_r", "bv_r", 0), ("xv_i", "wv_i", "bv_i", 1)]

            with tc.tile_pool(name="stream", bufs=2) as stream:
                for xn, wn, evacs in qk_projs:
                    x_sb = stream.tile([P, 8, S], b16, tag="x")
                    nc.sync.dma_start(
                        x_sb[:], x_in[xn].rearrange("(eo p) s -> p eo s", p=P))
                    w_sb = stream.tile([P, 8, DL], b16, tag="w")
                    nc.sync.dma_start(
                        w_sb[:], w_in[wn].rearrange("(eo p) d -> p eo d", p=P))
                    for dt in range(4):
                        ps = psp.tile([P, S], f32, tag="big")
                        for eo in range(8):
                            for nh in range(2):
                                nc.tensor.matmul(
                                    ps[:, nh * 512:(nh + 1) * 512],
                                    w_sb[:, eo, dt * P:(dt + 1) * P],
                                    x_sb[:, eo, nh * 512:(nh + 1) * 512],
                                    start=(eo == 0), stop=(eo == 7))
                        for half in range(2):
                            h = 2 * dt + half
                            src = ps[64 * half:64 * half + 64, :]
                            for dest, base, bn, sc in evacs:
                                b_ap = bias_sb[bn][64 * half:64 * half + 64,
                                                   dt:dt + 1]
                                nc.scalar.activation(
                                    dest[base:base + 64, h, :], src,
                                    Act.Identity, bias=b_ap, scale=sc)

                # ---- Phase 2: V projections (natural out) into VRIO
                for xn, wn, bn, ri in v_projs:
                    x_sb = stream.tile([P, 8, S], b16, tag="x")
                    nc.sync.dma_start(
                        x_sb[:], x_in[xn].rearrange("(eo p) s -> p eo s", p=P))
                    w_sb = stream.tile([P, 8, DL], b16, tag="w")
                    nc.sync.dma_start(
                        w_sb[:], w_in[wn].rearrange("(eo p) d -> p eo d", p=P))
                    for st in range(8):
                        psf = psp.tile([P, S], f32, tag="big")
                        ps = psf[:, 0:DL]
                        for eo in range(8):
                            nc.tensor.matmul(
                                ps, x_sb[:, eo, st * P:(st + 1) * P],
                                w_sb[:, eo, :],
                                start=(eo == 0), stop=(eo == 7))
                        nc.vector.tensor_tensor(
                            VRIO[:, st, :, ri * D:(ri + 1) * D],
                            ps.rearrange("p (h d) -> p h d", h=HLOC),
                            bias_sb[bn][:].rearrange("p (h d) -> p h d", h=HLOC),
                            Alu.add)

            # ---- Phase 3: attention, heads in pairs (ACT table batching)
            with (
                tc.tile_pool(name="s2p", bufs=3) as s2p,
                tc.tile_pool(name="ptp", bufs=2) as ptp,
                tc.tile_pool(name="sqp", bufs=3) as sqp,
            ):
                for hp in range(4):
                    pair = (2 * hp, 2 * hp + 1)
                    s2s = {}
                    for h in pair:
                        s2 = s2p.tile([P, 8, S], b16, tag="s2")
                        s2s[h] = s2
                        for kc in range(8):
                            for qh in range(2):
                                ps = psp.tile([P, S], f32, tag="big")
                                nc.tensor.matmul(
                                    ps[:, 0:512],
                                    KA_st[:, h, kc * P:(kc + 1) * P],
                                    R_st[:, h, qh * 512:(qh + 1) * 512],
                                    start=True, stop=True)
                                nc.tensor.matmul(
                                    ps[:, 512:1024],
                                    KA_st[:, h, kc * P:(kc + 1) * P],
                                    R2_st[:, h, qh * 512:(qh + 1) * 512],
                                    start=True, stop=True)
                                sq = sqp.tile([P, 1024], b16, tag="sq")
                                s2d = s2[:, kc, qh * 512:(qh + 1) * 512]
                                if kc * 2 + qh in ACT_SQ:
                                    nc.scalar.activation(sq[:], ps[:],
                                                         Act.Square)
                                    nc.gpsimd.tensor_tensor(
                                        s2d, sq[:, 0:512], sq[:, 512:1024],
                                        Alu.add)
                                else:
                                    nc.vector._custom_dve(
                                        _DVE_SQ["SQ_ANT2"],
                                        out=sq[:, 0:512], in0=ps[:, 0:512])
                                    nc.vector._custom_dve(
                                        _DVE_SQ["SQ_ADD_ANT2"],
                                        out=s2d, in0=ps[:, 512:1024],
                                        in1=sq[:, 0:512])
                    pts = {}
                    for h in pair:
                        nc.scalar.activation(
                            s2s[h][:].rearrange("p a c -> p (a c)"),
                            s2s[h][:].rearrange("p a c -> p (a c)"),
                            Act.Sqrt, scale=float(SCALE * SCALE))
                    for h in pair:
                        pt = ptp.tile([P, 8, S], b16, tag="pt")
                        pts[h] = pt
                        nc.scalar.activation(
                            pt[:].rearrange("p a c -> p (a c)"),
                            s2s[h][:].rearrange("p a c -> p (a c)"),
                            Act.Exp)
                    for h in pair:
                        for qt in range(8):
                            psa = pss.tile([P, 512], f32, tag="u")
                            for kc in range(8):
                                nc.tensor.matmul(
                                    psa[:, 0:129],
                                    pts[h][:, kc, qt * P:(qt + 1) * P],
                                    VRIO[:, kc, h, 0:129],
                                    start=(kc == 0), stop=(kc == 7))
                            rec = recp.tile([P, 1], f32, tag="rec")
                            nc.vector.reciprocal(rec[:], psa[:, 128:129])
                            # Att free layout: ri*512 + h*64 + d (real block,
                            # then imag block) so phase-4 transpose sources
                            # are contiguous [128,128] slices.
                            nc.vector.tensor_scalar(
                                Att[:, qt, :].rearrange(
                                    "p (a hh c) -> p a hh c",
                                    a=2, hh=HLOC)[:, :, h, :],
                                psa[:, 0:128].rearrange(
                                    "p (a c) -> p a c", a=2),
                                rec[:], None, op0=Alu.mult)
                    # ---- Phase 4 (pipelined): transpose this pair's att
                    # chunks (head dims back onto partitions)
                    for qt in range(8):
                        for ch in (hp, 4 + hp):
                            tpf = pss.tile([P, 1024], b16, tag="tsp")
                            tp = tpf[:, 0:P]
                            nc.tensor.transpose(
                                tp, Att[:, qt, ch * P:(ch + 1) * P],
                                ident[:])
                            nc.vector.tensor_copy(
                                AttT[:, ch, qt * P:(qt + 1) * P], tp)

            # ---- Phase 5: output projections (partial, no bias)
            with tc.tile_pool(name="fin", bufs=2) as fin:
                wo_sb = {}
                for n, ap in wo_in.items():
                    t = fin.tile([P, 4, E], b16, tag=n, bufs=1)
                    nc.sync.dma_start(t[:], ap.rearrange("(dc p) o -> p dc o", p=P))
                    wo_sb[n] = t
                for ri, (wn, on) in enumerate(
                        [("wo_r", "out_r"), ("wo_i", "out_i")]):
                    for st in range(8):
                        ob = fin.tile([P, E], f32, tag="ob")
                        for oh in range(2):
                            ps = pss.tile([P, DL], f32, tag="u")
                            for dc in range(4):
                                nc.tensor.matmul(
                                    ps[:],
                                    AttT[:, ri * 4 + dc, st * P:(st + 1) * P],
                                    wo_sb[wn][:, dc, oh * 512:(oh + 1) * 512],
                                    start=(dc == 0), stop=(dc == 3))
                            nc.scalar.activation(
                                ob[:, oh * 512:(oh + 1) * 512], ps[:],
                                Act.Identity)
                        nc.sync.dma_start(
                            out_d[on][st * P:(st + 1) * P, :], ob[:])

    nc.compile()
    return nc


def make_in_maps(inputs):
    """Shard + host-prep the full inputs into 8 per-core input maps."""
    inp = {k: np.asarray(v) for k, v in inputs.items()}
    xs = {
        "xq": ("query_real", "query_imag"),
        "xk": ("key_real", "key_imag"),
        "xv": ("value_real", "value_imag"),
    }
    per_g = []
    for g in range(2):
        rows = slice(g * DL, (g + 1) * DL)
        m = {}
        for wn, src in (("wq_r", "Wq_r"), ("wq_i", "Wq_i"),
                        ("wk_r", "Wk_r"), ("wk_i", "Wk_i"),
                        ("wv_r", "Wv_r"), ("wv_i", "Wv_i")):
            m[wn] = np.ascontiguousarray(inp[src][rows].T).astype(bf16)
        for wn, src in (("wo_r", "Wo_r"), ("wo_i", "Wo_i")):
            m[wn] = np.ascontiguousarray(inp[src][:, rows].T).astype(bf16)
        for bn, src in (("bq_r", "bq_r"), ("bq_i", "bq_i"),
                        ("bk_r", "bk_r"), ("bk_i", "bk_i")):
            m[bn] = np.ascontiguousarray(
                inp[src][rows].reshape(4, P).T).astype(np.float32)
        m["bq_rn"] = -m["bq_r"]
        for bn, src in (("bv_r", "bv_r"), ("bv_i", "bv_i")):
            m[bn] = np.ascontiguousarray(
                np.broadcast_to(inp[src][rows], (P, DL))).astype(np.float32)
        per_g.append(m)

    in_maps = []
    for c in range(8):
        b, g = c // 2, c % 2
        m = dict(per_g[g])
        for pref, (re_n, im_n) in xs.items():
            m[pref + "_r"] = np.ascontiguousarray(inp[re_n][b].T).astype(bf16)
            m[pref + "_i"] = np.ascontiguousarray(inp[im_n][b].T).astype(bf16)
        in_maps.append(m)
    return in_maps


def combine_outputs(results, inputs):
    bo_r = np.asarray(inputs["bo_r"], np.float32)
    bo_i = np.asarray(inputs["bo_i"], np.float32)
    B = 4
    out_r = np.empty((B, S, E), np.float32)
    out_i = np.empty((B, S, E), np.float32)
    for b in range(B):
        out_r[b] = results[2 * b]["out_r"] + results[2 * b + 1]["out_r"] + bo_r
        out_i[b] = results[2 * b]["out_i"] + results[2 * b + 1]["out_i"] + bo_i
    return out_r, out_i


def get_nc():
    global _NC_CACHE
    if _NC_CACHE is None:
        _NC_CACHE = _build()
    return _NC_CACHE


def kernel(**inputs):
    from concourse.bass_utils import run_bass_kernel_spmd

    nc = get_nc()
    in_maps = make_in_maps(inputs)
    res = run_bass_kernel_spmd(nc, in_maps, list(range(8)))
    return combine_outputs(res.results, inputs)
